# revision 36
# baseline (speedup 1.0000x reference)
"""Sliding-window causal self-attention with RoPE on 8 Trainium2 NeuronCores.

Problem: B=2, S=2048, D=1024, H=16, HD=64, WINDOW=256, fp32 in/out.
Sharding: 2 (batch) x 4 (head-groups of 4 heads). Each core computes its
head-group's QKV projections, RoPE, windowed attention, and a partial output
projection (y_g @ Wo_g.T); the host sums the 4 partials per batch.

v5: bf16 matmuls + global software pipeline + minimized DVE op count
(each DVE op carries ~200-300ns fixed cost, so ops are merged):
  - RoPE per projection chunk done 1024-wide: 2 ACT evacs into one raw
    tile, then stream_shuffle / mul / mul / add as four 1024-wide DVE ops.
    rotate_half sign is folded into the sin table (even rows negated).
  - mask add as ONE 4D-AP DVE op per 2-key-block scores group against a
    duplicated [128,2,384] mask; applied uniformly (edge garbage columns
    are never read by AV).
  - V accumulates into one 2-bank PSUM tile; evacuated with 2 strided ops.
  - denominators collected on partitions 0/32/64/96 of one tile, a single
    [128,512] reciprocal serves 4 AV chains; the last AV block uses the
    per-chain path to shorten the kernel tail.
  - scores+exp groups emitted one at a time inside phase 1 (as soon as
    their qf/kf columns exist) and between AV chains / Wo tiles, keeping
    the ACT exp pipeline drained and the PE stream dense.
  - PSUM: scores tile (2 banks) outer; phase-1 acc/vacc (6 banks) in an
    inner scope that closes to free banks for AV (4) and Wo (2).

Layout: feature-major ("T layout"); scores^T [k, q] per 128-key block over
a 384-query window; AV via lhsT=v_aug [128,65] (ones column ->
denominator); bf16 output, host upcasts + sums partials.
"""
import sys

for _p in ("/opt/trn_rl_repo", "/root/.axon_site/_ro/trn_rl_repo"):
    if _p not in sys.path:
        sys.path.append(_p)

import numpy as np
import ml_dtypes
import concourse.bacc as bacc
import concourse.mybir as mybir
from concourse.tile import TileContext
from concourse.bass_utils import run_bass_kernel_spmd

F32 = mybir.dt.float32
BF16 = mybir.dt.bfloat16
AF = mybir.ActivationFunctionType
NPBF = ml_dtypes.bfloat16

B, S, D = 2, 2048, 1024
H, HD = 16, 64
WINDOW = 256
THETA = 10000.0
SCALING = 1.0

HG = 4                      # head-groups (cores per batch)
HPG = H // HG               # heads per group = 4
GD = HPG * HD               # group out width = 256
NKB = S // 128              # 16 key blocks
SCALE = 1.0 / float(np.sqrt(HD))
MASKVAL = -240.0

_CACHE = {}
DEBUG = False

# pair-swap within each 32-lane group: out[2i] <-> in[2i+1]
_SWAP_MASK = [i ^ 1 for i in range(32)]


def _build():
    nc = bacc.Bacc(target_bir_lowering=False, trn_type="TRN2")

    xT = nc.dram_tensor("xT", [D, S], BF16, kind="ExternalInput")
    wq = nc.dram_tensor("wq", [128, D // 128 * GD], BF16, kind="ExternalInput")
    wk = nc.dram_tensor("wk", [128, D // 128 * GD], BF16, kind="ExternalInput")
    wv = nc.dram_tensor("wv", [128, D // 128 * GD], BF16, kind="ExternalInput")
    wo = nc.dram_tensor("wo", [128, GD // 128 * D], BF16, kind="ExternalInput")
    cos2 = nc.dram_tensor("cos2", [128, S], BF16, kind="ExternalInput")
    sin2 = nc.dram_tensor("sin2", [128, S], BF16, kind="ExternalInput")
    mask2 = nc.dram_tensor("mask2", [128, 768], F32, kind="ExternalInput")
    ones64 = nc.dram_tensor("ones64", [128, HPG * NKB], BF16, kind="ExternalInput")
    out = nc.dram_tensor("out", [S, D], BF16, kind="ExternalOutput")
    if DEBUG:
        d_qf = nc.dram_tensor("d_qf", [128, S], BF16, kind="ExternalOutput")
        d_kf = nc.dram_tensor("d_kf", [128, S], BF16, kind="ExternalOutput")
        d_v = nc.dram_tensor("d_v", [128, NKB * HPG * 65], BF16,
                             kind="ExternalOutput")
        d_attn = nc.dram_tensor("d_attn", [128, NKB * 384], BF16,
                                kind="ExternalOutput")
        d_yT = nc.dram_tensor("d_yT", [128, S], BF16, kind="ExternalOutput")

    with TileContext(nc) as tc:
        with tc.tile_pool(name="const", bufs=1) as cpool, \
             tc.tile_pool(name="persist", bufs=1) as ppool, \
             tc.tile_pool(name="attn", bufs=2) as apool, \
             tc.tile_pool(name="smalls", bufs=4) as spool, \
             tc.tile_pool(name="p3sb", bufs=3) as opool, \
             tc.tile_pool(name="scps", bufs=1, space="PSUM") as scps:
            wq_sb = cpool.tile([128, D // 128, GD], BF16)
            wk_sb = cpool.tile([128, D // 128, GD], BF16)
            wv_sb = cpool.tile([128, D // 128, GD], BF16)
            wo_sb = cpool.tile([128, GD // 128, D], BF16)
            cos_sb = cpool.tile([128, S], BF16)
            sin_sb = cpool.tile([128, S], BF16)
            mask_sb = cpool.tile([128, 2, 384], F32)
            nc.scalar.dma_start(
                wq_sb[:, 0:2, :].rearrange("p a b -> p (a b)"),
                wq.ap()[:, 0:2 * GD])
            nc.scalar.dma_start(
                wq_sb[:, 2:8, :].rearrange("p a b -> p (a b)"),
                wq.ap()[:, 2 * GD:8 * GD])
            nc.scalar.dma_start(wk_sb[:].rearrange("p a b -> p (a b)"), wk.ap())
            nc.scalar.dma_start(sin_sb[:], sin2[:])
            nc.scalar.dma_start(cos_sb[:], cos2[:])
            nc.scalar.dma_start(mask_sb[:].rearrange("p a b -> p (a b)"),
                                mask2.ap())

            v_sb = ppool.tile([128, NKB * HPG * 65], BF16)
            qf = [ppool.tile([128, S], BF16, name=f"qf{t}") for t in range(2)]
            kf = [ppool.tile([128, S], BF16, name=f"kf{t}") for t in range(2)]
            yT = [ppool.tile([128, S], BF16, name=f"yT{t}") for t in range(2)]

            attns_all = [
                [apool.tile([128, NKB * 384], BF16,
                            name=f"attn{th}_{i}", tag=f"attn{i}")
                 for i in range(2)]
                for th in range(2)]

            # ---- scores + exp emission (one group = 2 key blocks) -------
            sc_queue = [(th, i, 2 * p)
                        for p in range(8) for th in range(2) for i in range(2)]
            sc_pos = [0]

            def sc_group(th, i, kb0):
                ph = 64 * i
                sc = scps.tile([128, 2, 512], F32, tag="sc",
                               name=f"sc{th}_{kb0}_{i}")
                for g in range(2):
                    kb = kb0 + g
                    q0 = kb * 128
                    n = min(384, S - q0)
                    nc.tensor.matmul(sc[:, g, 0:n],
                                     kf[th][ph:ph + 64, q0:q0 + 128],
                                     qf[th][ph:ph + 64, q0:q0 + n],
                                     start=True, stop=True)
                scv = sc[:, :, 0:384].rearrange(
                    "p g (r c) -> p g r c", r=3)[:, :, 0::2, :]
                mkv = mask_sb[:].rearrange(
                    "p g (r c) -> p g r c", r=3)[:, :, 0::2, :]
                nc.vector.tensor_add(scv, scv, mkv)
                nc.scalar.activation(
                    attns_all[th][i][:, kb0 * 384:(kb0 + 2) * 384]
                    .rearrange("p (g c) -> p g c", g=2),
                    sc[:, :, 0:384], AF.Exp, scale=SCALE)

            def sc_next(n=1):
                for _ in range(n):
                    if sc_pos[0] < len(sc_queue):
                        sc_group(*sc_queue[sc_pos[0]])
                        sc_pos[0] += 1

            def sc_drain_until(k):
                while sc_pos[0] < k:
                    sc_next()

            # ---------------- phase 1: projections + rope ----------------
            HS = 1024
            with tc.tile_pool(name="p1x", bufs=1) as xpool, \
                 tc.tile_pool(name="p1raw", bufs=3) as rawpool, \
                 tc.tile_pool(name="p1tmp", bufs=3) as tpool, \
                 tc.tile_pool(name="p1ps", bufs=1, space="PSUM") as ps1:
                wsel = [(wq_sb, 0, qf[0]), (wq_sb, 128, qf[1]),
                        (wk_sb, 0, kf[0]), (wk_sb, 128, kf[1])]
                xrow = [[None] * (D // 128) for _ in range(2)]
                for half in range(2):
                    for kt in range(D // 128):
                        xrow[half][kt] = xpool.tile(
                            [128, HS], BF16, tag=f"x{half}_{kt}",
                            name=f"xrow{half}_{kt}")
                        nc.sync.dma_start(
                            xrow[half][kt][:],
                            xT.ap()[kt * 128:(kt + 1) * 128,
                                    half * HS:(half + 1) * HS])

                def rope_chunk(half, t, acc):
                    h0 = half * HS
                    dst = wsel[t][2]
                    raw2 = rawpool.tile([128, 1024], BF16, tag="raw")
                    nc.scalar.copy(raw2[:, 0:512], acc[0][:])
                    nc.scalar.copy(raw2[:, 512:1024], acc[1][:])
                    shf = tpool.tile([128, 1024], BF16, tag="shf")
                    nc.vector.stream_shuffle(shf[:], raw2[:], _SWAP_MASK)
                    t1 = tpool.tile([128, 1024], BF16, tag="t1")
                    nc.vector.tensor_mul(t1[:], shf[:], sin_sb[:, h0:h0 + HS])
                    t2 = tpool.tile([128, 1024], BF16, tag="t2")
                    nc.vector.tensor_mul(t2[:], raw2[:], cos_sb[:, h0:h0 + HS])
                    nc.vector.tensor_add(dst[:, h0:h0 + HS], t1[:], t2[:])

                for half in range(2):
                    pend = None
                    for t in range(4):
                        if half == 1:
                            sc_next(2)         # pre-drained groups per block
                        w_t, off, dst = wsel[t]
                        acc = [ps1.tile([128, 512], F32, name=f"acc{half}_{t}_{sl}",
                                        tag=f"acc{sl}", bufs=2) for sl in range(2)]
                        for kt in range(D // 128):
                            st, sp = (kt == 0), (kt == D // 128 - 1)
                            for sl in range(2):
                                nc.tensor.matmul(
                                    acc[sl][:], w_t[:, kt, off:off + 128],
                                    xrow[half][kt][:, sl * 512:(sl + 1) * 512],
                                    start=st, stop=sp)
                        if pend is not None:
                            rope_chunk(half, pend[0], pend[1])
                        pend = (t, acc)
                    if half == 0:
                        nc.scalar.dma_start(
                            wv_sb[:].rearrange("p a b -> p (a b)"), wv.ap())
                        nc.scalar.dma_start(
                            v_sb[:].rearrange("p (g c) -> p g c", c=65)[:, :, 64],
                            ones64[:])
                        nc.scalar.dma_start(
                            wo_sb[:].rearrange("p a b -> p (a b)"), wo.ap())
                    for g in range(2):
                        vacc = ps1.tile([128, 2, 512], F32, tag="vacc",
                                        name=f"vacc{half}_{g}", bufs=1)
                        for kt in range(D // 128):
                            st, sp = (kt == 0), (kt == D // 128 - 1)
                            for j in range(2):
                                for jj in range(2):
                                    stl = g * 4 + 2 * j + jj
                                    nc.tensor.matmul(
                                        vacc[:, j, jj * 256:(jj + 1) * 256],
                                        xrow[half][kt][:, stl * 128:(stl + 1) * 128],
                                        wv_sb[:, kt, 0:256],
                                        start=(st and jj == 0), stop=sp)
                        if pend is not None:
                            rope_chunk(half, pend[0], pend[1])
                            pend = None
                            sc_next(1 if half == 0 else 3)
                        else:
                            sc_next(1 if half == 0 else 3)
                        kb0v = half * 8 + g * 4
                        for j in range(2):
                            dstv = v_sb[:, (kb0v + 2 * j) * HPG * 65:
                                        (kb0v + 2 * j + 2) * HPG * 65]
                            nc.scalar.copy(
                                dstv.rearrange("p (kb h c) -> p kb h c",
                                               h=HPG, c=65)[:, :, :, 0:64],
                                vacc[:, j, :].rearrange(
                                    "p (kb h c) -> p kb h c", h=HPG, c=64))
                        sc_next(3 if half == 1 else 1)

            if DEBUG:
                nc.sync.dma_start(d_qf[:], qf[0][:])
                nc.sync.dma_start(d_kf[:], kf[0][:])
                nc.sync.dma_start(d_v[:], v_sb[:])

            # -------- phase 2+3: remaining scores, AV, Wo (interleaved) ---
            with tc.tile_pool(name="avps", bufs=4, space="PSUM") as avps, \
                 tc.tile_pool(name="p3ps", bufs=2, space="PSUM") as ps3:

                def av_chain(qq, th, i):
                    # one MM per contributing key block: widths up to 384
                    h = 2 * th + i
                    attn_h = attns_all[th][i]
                    acc = avps.tile([65, 512], F32, tag="av",
                                    name=f"av{th}_{i}_{qq}")
                    Q0 = qq * 512
                    kbs = [kb for kb in range(4 * qq - 2, 4 * qq + 4)
                           if 0 <= kb < NKB]
                    for idx, kb in enumerate(kbs):
                        q_lo = max(Q0, kb * 128)
                        q_hi = min(Q0 + 512, kb * 128 + 384, S)
                        ao = q_lo - kb * 128
                        w = q_hi - q_lo
                        nc.tensor.matmul(
                            acc[:, q_lo - Q0:q_lo - Q0 + w],
                            v_sb[:, (kb * HPG + h) * 65:
                                 (kb * HPG + h) * 65 + 65],
                            attn_h[:, kb * 384 + ao:kb * 384 + ao + w],
                            start=(idx == 0), stop=(idx == len(kbs) - 1))
                    return acc

                def wo_stile(stile):
                    r0 = stile * 128
                    ot = opool.tile([128, D], BF16, tag="ot")
                    for dc in range(2):
                        oacc = ps3.tile([128, 512], F32, tag="oacc")
                        for ct in range(2):
                            nc.tensor.matmul(
                                oacc[:], yT[ct][:, r0:r0 + 128],
                                wo_sb[:, ct, dc * 512:(dc + 1) * 512],
                                start=(ct == 0), stop=(ct == 1))
                        if dc == 0:
                            nc.scalar.copy(ot[:, 0:512], oacc[:])
                        else:
                            nc.vector.tensor_copy(ot[:, 512:1024], oacc[:])
                        nc.sync.dma_start(
                            out.ap()[r0:r0 + 128, dc * 512:(dc + 1) * 512],
                            ot[:, dc * 512:(dc + 1) * 512])

                TI = [(0, 0), (0, 1), (1, 0), (1, 1)]

                def av_block(qq):
                    # groups for pairs <= 2qq+1 must be emitted first
                    sc_drain_until(32 if qq >= 3 else 4 * (2 * qq + 2))
                    accs, rbss = [], []
                    # stage 1: chains + den/recip + EAGER gpsimd broadcast
                    for th, i in TI:
                        accs.append(av_chain(qq, th, i))
                        den = spool.tile([1, 512], F32, tag="dens")
                        nc.vector.tensor_copy(den[:], accs[-1][64:65, :])
                        rc = spool.tile([1, 512], F32, tag="rcs")
                        nc.vector.reciprocal_approx_fast(out=rc[:], in_=den[:])
                        rbs = spool.tile([32, 512], F32, tag="rbs")
                        nc.gpsimd.partition_broadcast(rbs[:], rc[0:1, :])
                        rbss.append(rbs)
                        sc_next()
                    # PE/ACT filler while the broadcasts drain
                    if qq >= 1:
                        wo_stile(4 * (qq - 1))
                        wo_stile(4 * (qq - 1) + 1)
                    # stage 3: normalize multiplies (two 32-partition ops
                    # sharing one broadcast tile)
                    for c, (th, i) in enumerate(TI):
                        for hq in range(2):
                            nc.vector.tensor_mul(
                                yT[th][64 * i + 32 * hq:64 * i + 32 * hq + 32,
                                       qq * 512:(qq + 1) * 512],
                                accs[c][32 * hq:32 * hq + 32, :], rbss[c][:])
                    if qq >= 1:
                        wo_stile(4 * (qq - 1) + 2)
                        wo_stile(4 * (qq - 1) + 3)

                for qq in range(4):
                    av_block(qq)
                for stile in range(8, 12):
                    wo_stile(stile)
                if DEBUG:
                    nc.sync.dma_start(d_attn[:], attns_all[0][0][:])
                    nc.sync.dma_start(d_yT[:], yT[0][:])
                for stile in range(12, 16):
                    wo_stile(stile)

    nc.finalize()
    return nc


def _rope_tables():
    inv_freq = 1.0 / (THETA ** (np.arange(0, HD, 2, dtype=np.float64) / HD))
    t = np.arange(S, dtype=np.float64) / max(SCALING, 1e-6)
    freqs = np.outer(t, inv_freq)                      # [S, HD/2]
    emb = np.concatenate((freqs, freqs), axis=-1)      # [S, HD]
    return np.cos(emb).astype(np.float32), np.sin(emb).astype(np.float32)


def _swz(w):
    kt = w.shape[0] // 128
    return np.ascontiguousarray(
        w.reshape(kt, 128, w.shape[1]).transpose(1, 0, 2).reshape(128, -1))


def _host_prep(x, Wq, Wk, Wv, Wo):
    cos, sin = _rope_tables()
    cosT2 = np.ascontiguousarray(np.tile(cos.T, (2, 1)))     # [128, S]
    sinT2 = np.ascontiguousarray(np.tile(sin.T, (2, 1)))
    sinT2[0::2, :] *= -1.0        # rotate_half sign folded into sin

    ii = np.arange(384)[None, :]
    jj = np.arange(128)[:, None]
    m = np.zeros((128, 384), dtype=np.float32)
    m[:, 0:128] += np.where(ii[:, 0:128] >= jj, 0.0, MASKVAL)
    m[:, 256:384] += np.where(ii[:, 256:384] - 256 < jj, 0.0, MASKVAL)
    m2 = np.ascontiguousarray(np.concatenate([m, m], axis=1))  # [128, 768]

    ones64 = np.ones((128, HPG * NKB), dtype=NPBF)

    in_maps = []
    for c in range(8):
        b, g = c // HG, c % HG
        gsl = slice(g * GD, (g + 1) * GD)
        in_maps.append({
            "xT": np.ascontiguousarray(x[b].T).astype(NPBF),
            "wq": _swz(Wq[gsl, :].T).astype(NPBF),
            "wk": _swz(Wk[gsl, :].T).astype(NPBF),
            "wv": _swz(Wv[gsl, :].T).astype(NPBF),
            "wo": _swz(Wo[:, gsl].T).astype(NPBF),
            "cos2": cosT2.astype(NPBF), "sin2": sinT2.astype(NPBF),
            "mask2": m2,
            "ones64": ones64,
        })
    return in_maps


def _run(inputs, trace=False, **kw):
    if "nc" not in _CACHE:
        _CACHE["nc"] = _build()
    in_maps = _host_prep(inputs["x"], inputs["Wq"], inputs["Wk"],
                         inputs["Wv"], inputs["Wo"])
    return run_bass_kernel_spmd(_CACHE["nc"], in_maps, list(range(8)),
                                trace=trace, **kw)


def kernel(x, Wq, Wk, Wv, Wo):
    res = _run({"x": x, "Wq": Wq, "Wk": Wk, "Wv": Wv, "Wo": Wo})
    out = np.zeros((B, S, D), dtype=np.float32)
    for c in range(8):
        out[c // HG] += np.asarray(res.results[c]["out"]).astype(np.float32)
    return out


# revision 37
# speedup vs baseline: 1.0155x; 1.0155x over previous
"""Sliding-window causal self-attention with RoPE on 8 Trainium2 NeuronCores.

Problem: B=2, S=2048, D=1024, H=16, HD=64, WINDOW=256, fp32 in/out.
Sharding: 2 (batch) x 4 (head-groups of 4 heads). Each core computes its
head-group's QKV projections, RoPE, windowed attention, and a partial output
projection (y_g @ Wo_g.T); the host sums the 4 partials per batch.

v5: bf16 matmuls + global software pipeline + minimized DVE op count
(each DVE op carries ~200-300ns fixed cost, so ops are merged):
  - RoPE per projection chunk done 1024-wide: 2 ACT evacs into one raw
    tile, then stream_shuffle / mul / mul / add as four 1024-wide DVE ops.
    rotate_half sign is folded into the sin table (even rows negated).
  - mask add as ONE 4D-AP DVE op per 2-key-block scores group against a
    duplicated [128,2,384] mask; applied uniformly (edge garbage columns
    are never read by AV).
  - V accumulates into one 2-bank PSUM tile; evacuated with 2 strided ops.
  - denominators collected on partitions 0/32/64/96 of one tile, a single
    [128,512] reciprocal serves 4 AV chains; the last AV block uses the
    per-chain path to shorten the kernel tail.
  - scores+exp groups emitted one at a time inside phase 1 (as soon as
    their qf/kf columns exist) and between AV chains / Wo tiles, keeping
    the ACT exp pipeline drained and the PE stream dense.
  - PSUM: scores tile (2 banks) outer; phase-1 acc/vacc (6 banks) in an
    inner scope that closes to free banks for AV (4) and Wo (2).

Layout: feature-major ("T layout"); scores^T [k, q] per 128-key block over
a 384-query window; AV via lhsT=v_aug [128,65] (ones column ->
denominator); bf16 output, host upcasts + sums partials.
"""
import sys

for _p in ("/opt/trn_rl_repo", "/root/.axon_site/_ro/trn_rl_repo"):
    if _p not in sys.path:
        sys.path.append(_p)

import numpy as np
import ml_dtypes
import concourse.bacc as bacc
import concourse.mybir as mybir
from concourse.tile import TileContext
from concourse.bass_utils import run_bass_kernel_spmd

F32 = mybir.dt.float32
BF16 = mybir.dt.bfloat16
AF = mybir.ActivationFunctionType
NPBF = ml_dtypes.bfloat16

B, S, D = 2, 2048, 1024
H, HD = 16, 64
WINDOW = 256
THETA = 10000.0
SCALING = 1.0

HG = 4                      # head-groups (cores per batch)
HPG = H // HG               # heads per group = 4
GD = HPG * HD               # group out width = 256
NKB = S // 128              # 16 key blocks
SCALE = 1.0 / float(np.sqrt(HD))
MASKVAL = -240.0

_CACHE = {}
DEBUG = False

# pair-swap within each 32-lane group: out[2i] <-> in[2i+1]
_SWAP_MASK = [i ^ 1 for i in range(32)]


def _build():
    nc = bacc.Bacc(target_bir_lowering=False, trn_type="TRN2")

    xT = nc.dram_tensor("xT", [D, S], BF16, kind="ExternalInput")
    wq = nc.dram_tensor("wq", [128, D // 128 * GD], BF16, kind="ExternalInput")
    wk = nc.dram_tensor("wk", [128, D // 128 * GD], BF16, kind="ExternalInput")
    wv = nc.dram_tensor("wv", [128, D // 128 * GD], BF16, kind="ExternalInput")
    wo = nc.dram_tensor("wo", [128, GD // 128 * D], BF16, kind="ExternalInput")
    cos2 = nc.dram_tensor("cos2", [128, S], BF16, kind="ExternalInput")
    sin2 = nc.dram_tensor("sin2", [128, S], BF16, kind="ExternalInput")
    mask2 = nc.dram_tensor("mask2", [128, 768], F32, kind="ExternalInput")
    ones64 = nc.dram_tensor("ones64", [128, HPG * NKB], BF16, kind="ExternalInput")
    out = nc.dram_tensor("out", [S, D], BF16, kind="ExternalOutput")
    if DEBUG:
        d_qf = nc.dram_tensor("d_qf", [128, S], BF16, kind="ExternalOutput")
        d_kf = nc.dram_tensor("d_kf", [128, S], BF16, kind="ExternalOutput")
        d_v = nc.dram_tensor("d_v", [128, NKB * HPG * 65], BF16,
                             kind="ExternalOutput")
        d_attn = nc.dram_tensor("d_attn", [128, NKB * 384], BF16,
                                kind="ExternalOutput")
        d_yT = nc.dram_tensor("d_yT", [128, S], BF16, kind="ExternalOutput")

    with TileContext(nc) as tc:
        with tc.tile_pool(name="const", bufs=1) as cpool, \
             tc.tile_pool(name="persist", bufs=1) as ppool, \
             tc.tile_pool(name="attn", bufs=2) as apool, \
             tc.tile_pool(name="smalls", bufs=4) as spool, \
             tc.tile_pool(name="p3sb", bufs=3) as opool, \
             tc.tile_pool(name="scps", bufs=1, space="PSUM") as scps:
            wq_sb = cpool.tile([128, D // 128, GD], BF16)
            wk_sb = cpool.tile([128, D // 128, GD], BF16)
            wv_sb = cpool.tile([128, D // 128, GD], BF16)
            wo_sb = cpool.tile([128, GD // 128, D], BF16)
            cos_sb = cpool.tile([128, S], BF16)
            sin_sb = cpool.tile([128, S], BF16)
            mask_sb = cpool.tile([128, 2, 384], F32)
            nc.scalar.dma_start(
                wq_sb[:, 0:2, :].rearrange("p a b -> p (a b)"),
                wq.ap()[:, 0:2 * GD])
            nc.scalar.dma_start(
                wq_sb[:, 2:8, :].rearrange("p a b -> p (a b)"),
                wq.ap()[:, 2 * GD:8 * GD])
            nc.scalar.dma_start(wk_sb[:].rearrange("p a b -> p (a b)"), wk.ap())
            nc.scalar.dma_start(sin_sb[:], sin2[:])
            nc.scalar.dma_start(cos_sb[:], cos2[:])
            nc.scalar.dma_start(mask_sb[:].rearrange("p a b -> p (a b)"),
                                mask2.ap())

            v_sb = ppool.tile([128, NKB * HPG * 65], BF16)
            qf = [ppool.tile([128, S], BF16, name=f"qf{t}") for t in range(2)]
            kf = [ppool.tile([128, S], BF16, name=f"kf{t}") for t in range(2)]
            yT = [ppool.tile([128, S], BF16, name=f"yT{t}") for t in range(2)]

            attns_all = [
                [apool.tile([128, NKB * 384], BF16,
                            name=f"attn{th}_{i}", tag=f"attn{i}")
                 for i in range(2)]
                for th in range(2)]

            # ---- scores + exp emission (one group = 2 key blocks) -------
            sc_queue = [(th, i, 2 * p)
                        for p in range(8) for th in range(2) for i in range(2)]
            sc_pos = [0]

            def sc_group(th, i, kb0):
                ph = 64 * i
                sc = scps.tile([128, 2, 512], F32, tag="sc",
                               name=f"sc{th}_{kb0}_{i}")
                for g in range(2):
                    kb = kb0 + g
                    q0 = kb * 128
                    n = min(384, S - q0)
                    nc.tensor.matmul(sc[:, g, 0:n],
                                     kf[th][ph:ph + 64, q0:q0 + 128],
                                     qf[th][ph:ph + 64, q0:q0 + n],
                                     start=True, stop=True)
                scv = sc[:, :, 0:384].rearrange(
                    "p g (r c) -> p g r c", r=3)[:, :, 0::2, :]
                mkv = mask_sb[:].rearrange(
                    "p g (r c) -> p g r c", r=3)[:, :, 0::2, :]
                nc.vector.tensor_add(scv, scv, mkv)
                nc.scalar.activation(
                    attns_all[th][i][:, kb0 * 384:(kb0 + 2) * 384]
                    .rearrange("p (g c) -> p g c", g=2),
                    sc[:, :, 0:384], AF.Exp, scale=SCALE)

            def sc_next(n=1):
                for _ in range(n):
                    if sc_pos[0] < len(sc_queue):
                        sc_group(*sc_queue[sc_pos[0]])
                        sc_pos[0] += 1

            def sc_drain_until(k):
                while sc_pos[0] < k:
                    sc_next()

            # ---------------- phase 1: projections + rope ----------------
            HS = 1024
            with tc.tile_pool(name="p1x", bufs=1) as xpool, \
                 tc.tile_pool(name="p1raw", bufs=3) as rawpool, \
                 tc.tile_pool(name="p1tmp", bufs=3) as tpool, \
                 tc.tile_pool(name="p1ps", bufs=1, space="PSUM") as ps1:
                wsel = [(wq_sb, 0, qf[0]), (wq_sb, 128, qf[1]),
                        (wk_sb, 0, kf[0]), (wk_sb, 128, kf[1])]
                xrow = [[None] * (D // 128) for _ in range(2)]
                for half in range(2):
                    for kt in range(D // 128):
                        xrow[half][kt] = xpool.tile(
                            [128, HS], BF16, tag=f"x{half}_{kt}",
                            name=f"xrow{half}_{kt}")
                        nc.sync.dma_start(
                            xrow[half][kt][:],
                            xT.ap()[kt * 128:(kt + 1) * 128,
                                    half * HS:(half + 1) * HS])

                def rope_chunk(half, t, acc):
                    h0 = half * HS
                    dst = wsel[t][2]
                    raw2 = rawpool.tile([128, 1024], BF16, tag="raw")
                    nc.scalar.copy(raw2[:, 0:512], acc[0][:])
                    nc.scalar.copy(raw2[:, 512:1024], acc[1][:])
                    shf = tpool.tile([128, 1024], BF16, tag="shf")
                    nc.vector.stream_shuffle(shf[:], raw2[:], _SWAP_MASK)
                    t1 = tpool.tile([128, 1024], BF16, tag="t1")
                    nc.vector.tensor_mul(t1[:], shf[:], sin_sb[:, h0:h0 + HS])
                    t2 = tpool.tile([128, 1024], BF16, tag="t2")
                    nc.vector.tensor_mul(t2[:], raw2[:], cos_sb[:, h0:h0 + HS])
                    nc.vector.tensor_add(dst[:, h0:h0 + HS], t1[:], t2[:])

                for half in range(2):
                    pend = None
                    for t in range(4):
                        if half == 1:
                            sc_next(2)         # pre-drained groups per block
                        w_t, off, dst = wsel[t]
                        acc = [ps1.tile([128, 512], F32, name=f"acc{half}_{t}_{sl}",
                                        tag=f"acc{sl}", bufs=2) for sl in range(2)]
                        for kt in range(D // 128):
                            st, sp = (kt == 0), (kt == D // 128 - 1)
                            for sl in range(2):
                                nc.tensor.matmul(
                                    acc[sl][:], w_t[:, kt, off:off + 128],
                                    xrow[half][kt][:, sl * 512:(sl + 1) * 512],
                                    start=st, stop=sp)
                        if pend is not None:
                            rope_chunk(half, pend[0], pend[1])
                        pend = (t, acc)
                    if half == 0:
                        nc.scalar.dma_start(
                            wv_sb[:].rearrange("p a b -> p (a b)"), wv.ap())
                        nc.scalar.dma_start(
                            v_sb[:].rearrange("p (g c) -> p g c", c=65)[:, :, 64],
                            ones64[:])
                        nc.scalar.dma_start(
                            wo_sb[:].rearrange("p a b -> p (a b)"), wo.ap())
                    for g in range(2):
                        vacc = ps1.tile([128, 2, 512], F32, tag="vacc",
                                        name=f"vacc{half}_{g}", bufs=1)
                        for kt in range(D // 128):
                            st, sp = (kt == 0), (kt == D // 128 - 1)
                            for j in range(2):
                                for jj in range(2):
                                    stl = g * 4 + 2 * j + jj
                                    nc.tensor.matmul(
                                        vacc[:, j, jj * 256:(jj + 1) * 256],
                                        xrow[half][kt][:, stl * 128:(stl + 1) * 128],
                                        wv_sb[:, kt, 0:256],
                                        start=(st and jj == 0), stop=sp)
                        if pend is not None:
                            rope_chunk(half, pend[0], pend[1])
                            pend = None
                            sc_next(1 if half == 0 else 3)
                        else:
                            sc_next(1 if half == 0 else 3)
                        kb0v = half * 8 + g * 4
                        for j in range(2):
                            dstv = v_sb[:, (kb0v + 2 * j) * HPG * 65:
                                        (kb0v + 2 * j + 2) * HPG * 65]
                            nc.scalar.copy(
                                dstv.rearrange("p (kb h c) -> p kb h c",
                                               h=HPG, c=65)[:, :, :, 0:64],
                                vacc[:, j, :].rearrange(
                                    "p (kb h c) -> p kb h c", h=HPG, c=64))
                        sc_next(3 if half == 1 else 1)

            if DEBUG:
                nc.sync.dma_start(d_qf[:], qf[0][:])
                nc.sync.dma_start(d_kf[:], kf[0][:])
                nc.sync.dma_start(d_v[:], v_sb[:])

            # -------- phase 2+3: remaining scores, AV, Wo (interleaved) ---
            with tc.tile_pool(name="avps", bufs=4, space="PSUM") as avps, \
                 tc.tile_pool(name="p3ps", bufs=2, space="PSUM") as ps3:

                def av_chain(qq, th, i):
                    # one MM per contributing key block: widths up to 384
                    h = 2 * th + i
                    attn_h = attns_all[th][i]
                    acc = avps.tile([65, 512], F32, tag="av",
                                    name=f"av{th}_{i}_{qq}")
                    Q0 = qq * 512
                    kbs = [kb for kb in range(4 * qq - 2, 4 * qq + 4)
                           if 0 <= kb < NKB]
                    for idx, kb in enumerate(kbs):
                        q_lo = max(Q0, kb * 128)
                        q_hi = min(Q0 + 512, kb * 128 + 384, S)
                        ao = q_lo - kb * 128
                        w = q_hi - q_lo
                        nc.tensor.matmul(
                            acc[:, q_lo - Q0:q_lo - Q0 + w],
                            v_sb[:, (kb * HPG + h) * 65:
                                 (kb * HPG + h) * 65 + 65],
                            attn_h[:, kb * 384 + ao:kb * 384 + ao + w],
                            start=(idx == 0), stop=(idx == len(kbs) - 1))
                    return acc

                def wo_stile(stile):
                    r0 = stile * 128
                    ot = opool.tile([128, D], BF16, tag="ot")
                    for dc in range(2):
                        oacc = ps3.tile([128, 512], F32, tag="oacc")
                        for ct in range(2):
                            nc.tensor.matmul(
                                oacc[:], yT[ct][:, r0:r0 + 128],
                                wo_sb[:, ct, dc * 512:(dc + 1) * 512],
                                start=(ct == 0), stop=(ct == 1))
                        if dc == 0:
                            nc.scalar.copy(ot[:, 0:512], oacc[:])
                        else:
                            nc.vector.tensor_copy(ot[:, 512:1024], oacc[:])
                        nc.sync.dma_start(
                            out.ap()[r0:r0 + 128, dc * 512:(dc + 1) * 512],
                            ot[:, dc * 512:(dc + 1) * 512])

                TI = [(0, 0), (0, 1), (1, 0), (1, 1)]

                def av_block(qq):
                    # groups for pairs <= 2qq+1 must be emitted first
                    sc_drain_until(32 if qq >= 3 else 4 * (2 * qq + 2))
                    accs, rbss = [], []
                    # stage 1: chains + den/recip + EAGER gpsimd broadcast
                    for th, i in TI:
                        accs.append(av_chain(qq, th, i))
                        den = spool.tile([1, 512], F32, tag="dens")
                        nc.vector.tensor_copy(den[:], accs[-1][64:65, :])
                        rc = spool.tile([1, 512], F32, tag="rcs")
                        nc.vector.reciprocal_approx_fast(out=rc[:], in_=den[:])
                        rbs = spool.tile([64, 512], F32, tag="rbs")
                        nc.gpsimd.partition_broadcast(rbs[:], rc[0:1, :])
                        rbss.append(rbs)
                        sc_next()
                    # PE/ACT filler while the broadcasts drain
                    if qq >= 1:
                        wo_stile(4 * (qq - 1))
                        wo_stile(4 * (qq - 1) + 1)
                    # stage 3: normalize multiplies
                    for c, (th, i) in enumerate(TI):
                        nc.vector.tensor_mul(
                            yT[th][64 * i:64 * i + 64,
                                   qq * 512:(qq + 1) * 512],
                            accs[c][0:64, :], rbss[c][:])
                    if qq >= 1:
                        wo_stile(4 * (qq - 1) + 2)
                        wo_stile(4 * (qq - 1) + 3)

                for qq in range(4):
                    av_block(qq)
                for stile in range(8, 12):
                    wo_stile(stile)
                if DEBUG:
                    nc.sync.dma_start(d_attn[:], attns_all[0][0][:])
                    nc.sync.dma_start(d_yT[:], yT[0][:])
                for stile in range(12, 16):
                    wo_stile(stile)

    nc.finalize()
    return nc


def _rope_tables():
    inv_freq = 1.0 / (THETA ** (np.arange(0, HD, 2, dtype=np.float64) / HD))
    t = np.arange(S, dtype=np.float64) / max(SCALING, 1e-6)
    freqs = np.outer(t, inv_freq)                      # [S, HD/2]
    emb = np.concatenate((freqs, freqs), axis=-1)      # [S, HD]
    return np.cos(emb).astype(np.float32), np.sin(emb).astype(np.float32)


def _swz(w):
    kt = w.shape[0] // 128
    return np.ascontiguousarray(
        w.reshape(kt, 128, w.shape[1]).transpose(1, 0, 2).reshape(128, -1))


def _host_prep(x, Wq, Wk, Wv, Wo):
    cos, sin = _rope_tables()
    cosT2 = np.ascontiguousarray(np.tile(cos.T, (2, 1)))     # [128, S]
    sinT2 = np.ascontiguousarray(np.tile(sin.T, (2, 1)))
    sinT2[0::2, :] *= -1.0        # rotate_half sign folded into sin

    ii = np.arange(384)[None, :]
    jj = np.arange(128)[:, None]
    m = np.zeros((128, 384), dtype=np.float32)
    m[:, 0:128] += np.where(ii[:, 0:128] >= jj, 0.0, MASKVAL)
    m[:, 256:384] += np.where(ii[:, 256:384] - 256 < jj, 0.0, MASKVAL)
    m2 = np.ascontiguousarray(np.concatenate([m, m], axis=1))  # [128, 768]

    ones64 = np.ones((128, HPG * NKB), dtype=NPBF)

    in_maps = []
    for c in range(8):
        b, g = c // HG, c % HG
        gsl = slice(g * GD, (g + 1) * GD)
        in_maps.append({
            "xT": np.ascontiguousarray(x[b].T).astype(NPBF),
            "wq": _swz(Wq[gsl, :].T).astype(NPBF),
            "wk": _swz(Wk[gsl, :].T).astype(NPBF),
            "wv": _swz(Wv[gsl, :].T).astype(NPBF),
            "wo": _swz(Wo[:, gsl].T).astype(NPBF),
            "cos2": cosT2.astype(NPBF), "sin2": sinT2.astype(NPBF),
            "mask2": m2,
            "ones64": ones64,
        })
    return in_maps


def _run(inputs, trace=False, **kw):
    if "nc" not in _CACHE:
        _CACHE["nc"] = _build()
    in_maps = _host_prep(inputs["x"], inputs["Wq"], inputs["Wk"],
                         inputs["Wv"], inputs["Wo"])
    return run_bass_kernel_spmd(_CACHE["nc"], in_maps, list(range(8)),
                                trace=trace, **kw)


def kernel(x, Wq, Wk, Wv, Wo):
    res = _run({"x": x, "Wq": Wq, "Wk": Wk, "Wv": Wv, "Wo": Wo})
    out = np.zeros((B, S, D), dtype=np.float32)
    for c in range(8):
        out[c // HG] += np.asarray(res.results[c]["out"]).astype(np.float32)
    return out


# revision 40
# speedup vs baseline: 1.0937x; 1.0770x over previous
"""Sliding-window causal self-attention with RoPE on 8 Trainium2 NeuronCores.

Problem: B=2, S=2048, D=1024, H=16, HD=64, WINDOW=256, fp32 in/out.
Sharding: 2 (batch) x 4 (head-groups of 4 heads). Each core computes its
head-group's QKV projections, RoPE, windowed attention, and a partial output
projection (y_g @ Wo_g.T); the host sums the 4 partials per batch.

v5: bf16 matmuls + global software pipeline + minimized DVE op count
(each DVE op carries ~200-300ns fixed cost, so ops are merged):
  - RoPE per projection chunk done 1024-wide: 2 ACT evacs into one raw
    tile, then stream_shuffle / mul / mul / add as four 1024-wide DVE ops.
    rotate_half sign is folded into the sin table (even rows negated).
  - mask add as ONE 4D-AP DVE op per 2-key-block scores group against a
    duplicated [128,2,384] mask; applied uniformly (edge garbage columns
    are never read by AV).
  - V accumulates into one 2-bank PSUM tile; evacuated with 2 strided ops.
  - denominators collected on partitions 0/32/64/96 of one tile, a single
    [128,512] reciprocal serves 4 AV chains; the last AV block uses the
    per-chain path to shorten the kernel tail.
  - scores+exp groups emitted one at a time inside phase 1 (as soon as
    their qf/kf columns exist) and between AV chains / Wo tiles, keeping
    the ACT exp pipeline drained and the PE stream dense.
  - PSUM: scores tile (2 banks) outer; phase-1 acc/vacc (6 banks) in an
    inner scope that closes to free banks for AV (4) and Wo (2).

Layout: feature-major ("T layout"); scores^T [k, q] per 128-key block over
a 384-query window; AV via lhsT=v_aug [128,65] (ones column ->
denominator); bf16 output, host upcasts + sums partials.
"""
import sys

for _p in ("/opt/trn_rl_repo", "/root/.axon_site/_ro/trn_rl_repo"):
    if _p not in sys.path:
        sys.path.append(_p)

import numpy as np
import ml_dtypes
import concourse.bacc as bacc
import concourse.mybir as mybir
from concourse.tile import TileContext
from concourse.bass_utils import run_bass_kernel_spmd

F32 = mybir.dt.float32
BF16 = mybir.dt.bfloat16
AF = mybir.ActivationFunctionType
NPBF = ml_dtypes.bfloat16

B, S, D = 2, 2048, 1024
H, HD = 16, 64
WINDOW = 256
THETA = 10000.0
SCALING = 1.0

HG = 4                      # head-groups (cores per batch)
HPG = H // HG               # heads per group = 4
GD = HPG * HD               # group out width = 256
NKB = S // 128              # 16 key blocks
SCALE = 1.0 / float(np.sqrt(HD))
MASKVAL = -240.0

_CACHE = {}
DEBUG = False

# pair-swap within each 32-lane group: out[2i] <-> in[2i+1]
_SWAP_MASK = [i ^ 1 for i in range(32)]


def _build():
    nc = bacc.Bacc(target_bir_lowering=False, trn_type="TRN2")

    xT = nc.dram_tensor("xT", [D, S], BF16, kind="ExternalInput")
    wq = nc.dram_tensor("wq", [128, D // 128 * GD], BF16, kind="ExternalInput")
    wk = nc.dram_tensor("wk", [128, D // 128 * GD], BF16, kind="ExternalInput")
    wv = nc.dram_tensor("wv", [128, D // 128 * GD], BF16, kind="ExternalInput")
    wo = nc.dram_tensor("wo", [128, GD // 128 * D], BF16, kind="ExternalInput")
    cos2 = nc.dram_tensor("cos2", [128, S], BF16, kind="ExternalInput")
    sin2 = nc.dram_tensor("sin2", [128, S], BF16, kind="ExternalInput")
    mask2 = nc.dram_tensor("mask2", [128, 768], F32, kind="ExternalInput")
    ones64 = nc.dram_tensor("ones64", [128, HPG * NKB], BF16, kind="ExternalInput")
    out = nc.dram_tensor("out", [S, D], BF16, kind="ExternalOutput")
    if DEBUG:
        d_qf = nc.dram_tensor("d_qf", [128, S], BF16, kind="ExternalOutput")
        d_kf = nc.dram_tensor("d_kf", [128, S], BF16, kind="ExternalOutput")
        d_v = nc.dram_tensor("d_v", [128, NKB * HPG * 65], BF16,
                             kind="ExternalOutput")
        d_attn = nc.dram_tensor("d_attn", [128, NKB * 384], BF16,
                                kind="ExternalOutput")
        d_yT = nc.dram_tensor("d_yT", [128, S], BF16, kind="ExternalOutput")

    with TileContext(nc) as tc:
        with tc.tile_pool(name="const", bufs=1) as cpool, \
             tc.tile_pool(name="persist", bufs=1) as ppool, \
             tc.tile_pool(name="attn", bufs=2) as apool, \
             tc.tile_pool(name="smalls", bufs=4) as spool, \
             tc.tile_pool(name="p3sb", bufs=3) as opool, \
             tc.tile_pool(name="scps", bufs=1, space="PSUM") as scps:
            wq_sb = cpool.tile([128, D // 128, GD], BF16)
            wk_sb = cpool.tile([128, D // 128, GD], BF16)
            wv_sb = cpool.tile([128, D // 128, GD], BF16)
            wo_sb = cpool.tile([128, GD // 128, D], BF16)
            cos_sb = cpool.tile([128, S], BF16)
            sin_sb = cpool.tile([128, S], BF16)
            mask_sb = cpool.tile([128, 2, 384], F32)
            nc.scalar.dma_start(
                wq_sb[:, 0:2, :].rearrange("p a b -> p (a b)"),
                wq.ap()[:, 0:2 * GD])
            nc.scalar.dma_start(
                wq_sb[:, 2:8, :].rearrange("p a b -> p (a b)"),
                wq.ap()[:, 2 * GD:8 * GD])
            nc.scalar.dma_start(wk_sb[:].rearrange("p a b -> p (a b)"), wk.ap())
            nc.scalar.dma_start(sin_sb[:], sin2[:])
            nc.scalar.dma_start(cos_sb[:], cos2[:])
            nc.scalar.dma_start(mask_sb[:].rearrange("p a b -> p (a b)"),
                                mask2.ap())

            v_sb = ppool.tile([128, NKB * HPG * 65], BF16)
            qf = [ppool.tile([128, S], BF16, name=f"qf{t}") for t in range(2)]
            kf = [ppool.tile([128, S], BF16, name=f"kf{t}") for t in range(2)]
            yT = [ppool.tile([128, S], BF16, name=f"yT{t}") for t in range(2)]

            attns_all = [
                [apool.tile([128, NKB * 384], BF16,
                            name=f"attn{th}_{i}", tag=f"attn{i}")
                 for i in range(2)]
                for th in range(2)]

            # ---- scores + exp emission (one group = 2 key blocks) -------
            sc_queue = [(th, i, 2 * p)
                        for p in range(8) for th in range(2) for i in range(2)]
            sc_pos = [0]

            def sc_group(th, i, kb0):
                ph = 64 * i
                sc = scps.tile([128, 2, 512], F32, tag="sc",
                               name=f"sc{th}_{kb0}_{i}")
                for g in range(2):
                    kb = kb0 + g
                    q0 = kb * 128
                    n = min(384, S - q0)
                    nc.tensor.matmul(sc[:, g, 0:n],
                                     kf[th][ph:ph + 64, q0:q0 + 128],
                                     qf[th][ph:ph + 64, q0:q0 + n],
                                     start=True, stop=True)
                scv = sc[:, :, 0:384].rearrange(
                    "p g (r c) -> p g r c", r=3)[:, :, 0::2, :]
                mkv = mask_sb[:].rearrange(
                    "p g (r c) -> p g r c", r=3)[:, :, 0::2, :]
                nc.vector.tensor_add(scv, scv, mkv)
                nc.scalar.activation(
                    attns_all[th][i][:, kb0 * 384:(kb0 + 2) * 384]
                    .rearrange("p (g c) -> p g c", g=2),
                    sc[:, :, 0:384], AF.Exp, scale=SCALE)

            def sc_next(n=1):
                for _ in range(n):
                    if sc_pos[0] < len(sc_queue):
                        sc_group(*sc_queue[sc_pos[0]])
                        sc_pos[0] += 1

            def sc_drain_until(k):
                while sc_pos[0] < k:
                    sc_next()

            # ---------------- phase 1: projections + rope ----------------
            HS = 1024
            with tc.tile_pool(name="p1x", bufs=1) as xpool, \
                 tc.tile_pool(name="p1raw", bufs=3) as rawpool, \
                 tc.tile_pool(name="p1tmp", bufs=3) as tpool, \
                 tc.tile_pool(name="p1ps", bufs=1, space="PSUM") as ps1:
                wsel = [(wq_sb, 0, qf[0]), (wq_sb, 128, qf[1]),
                        (wk_sb, 0, kf[0]), (wk_sb, 128, kf[1])]
                xrow = [[None] * (D // 128) for _ in range(2)]
                for half in range(2):
                    for kt in range(D // 128):
                        xrow[half][kt] = xpool.tile(
                            [128, HS], BF16, tag=f"x{half}_{kt}",
                            name=f"xrow{half}_{kt}")
                        nc.sync.dma_start(
                            xrow[half][kt][:],
                            xT.ap()[kt * 128:(kt + 1) * 128,
                                    half * HS:(half + 1) * HS])

                def rope_chunk(half, t, acc):
                    h0 = half * HS
                    dst = wsel[t][2]
                    raw2 = rawpool.tile([128, 1024], BF16, tag="raw")
                    nc.scalar.copy(raw2[:, 0:512], acc[0][:])
                    nc.scalar.copy(raw2[:, 512:1024], acc[1][:])
                    shf = tpool.tile([128, 1024], BF16, tag="shf")
                    nc.vector.stream_shuffle(shf[:], raw2[:], _SWAP_MASK)
                    t1 = tpool.tile([128, 1024], BF16, tag="t1")
                    nc.vector.tensor_mul(t1[:], shf[:], sin_sb[:, h0:h0 + HS])
                    t2 = tpool.tile([128, 1024], BF16, tag="t2")
                    nc.vector.tensor_mul(t2[:], raw2[:], cos_sb[:, h0:h0 + HS])
                    nc.vector.tensor_add(dst[:, h0:h0 + HS], t1[:], t2[:])

                for half in range(2):
                    pend = None
                    for t in range(4):
                        if half == 1:
                            sc_next(2)         # pre-drained groups per block
                        w_t, off, dst = wsel[t]
                        acc = [ps1.tile([128, 512], F32, name=f"acc{half}_{t}_{sl}",
                                        tag=f"acc{sl}", bufs=2) for sl in range(2)]
                        for kt in range(D // 128):
                            st, sp = (kt == 0), (kt == D // 128 - 1)
                            for sl in range(2):
                                nc.tensor.matmul(
                                    acc[sl][:], w_t[:, kt, off:off + 128],
                                    xrow[half][kt][:, sl * 512:(sl + 1) * 512],
                                    start=st, stop=sp)
                        if pend is not None:
                            rope_chunk(half, pend[0], pend[1])
                        pend = (t, acc)
                    if half == 0:
                        nc.scalar.dma_start(
                            wv_sb[:].rearrange("p a b -> p (a b)"), wv.ap())
                        nc.scalar.dma_start(
                            v_sb[:].rearrange("p (g c) -> p g c", c=65)[:, :, 64],
                            ones64[:])
                        nc.scalar.dma_start(
                            wo_sb[:].rearrange("p a b -> p (a b)"), wo.ap())
                    for g in range(2):
                        vacc = ps1.tile([128, 2, 512], F32, tag="vacc",
                                        name=f"vacc{half}_{g}", bufs=1)
                        for kt in range(D // 128):
                            st, sp = (kt == 0), (kt == D // 128 - 1)
                            for j in range(2):
                                for jj in range(2):
                                    stl = g * 4 + 2 * j + jj
                                    nc.tensor.matmul(
                                        vacc[:, j, jj * 256:(jj + 1) * 256],
                                        xrow[half][kt][:, stl * 128:(stl + 1) * 128],
                                        wv_sb[:, kt, 0:256],
                                        start=(st and jj == 0), stop=sp)
                        if pend is not None:
                            rope_chunk(half, pend[0], pend[1])
                            pend = None
                            sc_next(1 if half == 0 else 3)
                        else:
                            sc_next(1 if half == 0 else 3)
                        kb0v = half * 8 + g * 4
                        for j in range(2):
                            dstv = v_sb[:, (kb0v + 2 * j) * HPG * 65:
                                        (kb0v + 2 * j + 2) * HPG * 65]
                            nc.scalar.copy(
                                dstv.rearrange("p (kb h c) -> p kb h c",
                                               h=HPG, c=65)[:, :, :, 0:64],
                                vacc[:, j, :].rearrange(
                                    "p (kb h c) -> p kb h c", h=HPG, c=64))
                        sc_next(3 if half == 1 else 1)

            if DEBUG:
                nc.sync.dma_start(d_qf[:], qf[0][:])
                nc.sync.dma_start(d_kf[:], kf[0][:])
                nc.sync.dma_start(d_v[:], v_sb[:])

            # -------- phase 2+3: remaining scores, AV, Wo (interleaved) ---
            with tc.tile_pool(name="avps", bufs=4, space="PSUM") as avps, \
                 tc.tile_pool(name="p3ps", bufs=2, space="PSUM") as ps3:

                def av_chain(qq, th, i):
                    # one MM per contributing key block: widths up to 384
                    h = 2 * th + i
                    attn_h = attns_all[th][i]
                    acc = avps.tile([65, 512], F32, tag="av",
                                    name=f"av{th}_{i}_{qq}")
                    Q0 = qq * 512
                    kbs = [kb for kb in range(4 * qq - 2, 4 * qq + 4)
                           if 0 <= kb < NKB]
                    for idx, kb in enumerate(kbs):
                        q_lo = max(Q0, kb * 128)
                        q_hi = min(Q0 + 512, kb * 128 + 384, S)
                        ao = q_lo - kb * 128
                        w = q_hi - q_lo
                        nc.tensor.matmul(
                            acc[:, q_lo - Q0:q_lo - Q0 + w],
                            v_sb[:, (kb * HPG + h) * 65:
                                 (kb * HPG + h) * 65 + 65],
                            attn_h[:, kb * 384 + ao:kb * 384 + ao + w],
                            start=(idx == 0), stop=(idx == len(kbs) - 1))
                    return acc

                def wo_stile(stile):
                    r0 = stile * 128
                    ot = opool.tile([128, D], BF16, tag="ot")
                    for dc in range(2):
                        oacc = ps3.tile([128, 512], F32, tag="oacc")
                        for ct in range(2):
                            nc.tensor.matmul(
                                oacc[:], yT[ct][:, r0:r0 + 128],
                                wo_sb[:, ct, dc * 512:(dc + 1) * 512],
                                start=(ct == 0), stop=(ct == 1))
                        if dc == 0:
                            nc.scalar.copy(ot[:, 0:512], oacc[:])
                        else:
                            nc.vector.tensor_copy(ot[:, 512:1024], oacc[:])
                        nc.sync.dma_start(
                            out.ap()[r0:r0 + 128, dc * 512:(dc + 1) * 512],
                            ot[:, dc * 512:(dc + 1) * 512])

                TI = [(0, 0), (0, 1), (1, 0), (1, 1)]

                def av_block(qq):
                    # groups for pairs <= 2qq+1 must be emitted first
                    sc_drain_until(32 if qq >= 3 else 4 * (2 * qq + 2))
                    accs, rbss = [], []
                    # stage 1: chains + den/recip + EAGER gpsimd broadcast
                    for th, i in TI:
                        accs.append(av_chain(qq, th, i))
                        den = spool.tile([1, 512], F32, tag="dens")
                        nc.vector.tensor_copy(den[:], accs[-1][64:65, :])
                        rc = spool.tile([1, 512], F32, tag="rcs")
                        nc.vector.reciprocal_approx_fast(out=rc[:], in_=den[:])
                        rbs = spool.tile([64, 512], F32, tag="rbs")
                        nc.gpsimd.partition_broadcast(rbs[:], rc[0:1, :])
                        rbss.append(rbs)
                        sc_next()
                    # PE/ACT filler while the broadcasts drain
                    if qq >= 1:
                        wo_stile(4 * (qq - 1))
                        wo_stile(4 * (qq - 1) + 1)
                    # stage 3: normalize multiplies; the last block splits
                    # by column half so the final Wo tiles start as soon as
                    # their yT columns exist
                    if qq < 3:
                        for c, (th, i) in enumerate(TI):
                            nc.vector.tensor_mul(
                                yT[th][64 * i:64 * i + 64,
                                       qq * 512:(qq + 1) * 512],
                                accs[c][0:64, :], rbss[c][:])
                        if qq >= 1:
                            wo_stile(4 * (qq - 1) + 2)
                            wo_stile(4 * (qq - 1) + 3)
                    else:
                        for half in range(2):
                            cl = qq * 512 + half * 256
                            for c, (th, i) in enumerate(TI):
                                nc.vector.tensor_mul(
                                    yT[th][64 * i:64 * i + 64, cl:cl + 256],
                                    accs[c][0:64, half * 256:half * 256 + 256],
                                    rbss[c][:, half * 256:half * 256 + 256])
                            if half == 0:
                                wo_stile(10)
                                wo_stile(11)
                                wo_stile(12)
                                wo_stile(13)
                            else:
                                wo_stile(14)
                                wo_stile(15)

                for qq in range(4):
                    av_block(qq)
                if DEBUG:
                    nc.sync.dma_start(d_attn[:], attns_all[0][0][:])
                    nc.sync.dma_start(d_yT[:], yT[0][:])

    nc.finalize()
    return nc


def _rope_tables():
    inv_freq = 1.0 / (THETA ** (np.arange(0, HD, 2, dtype=np.float64) / HD))
    t = np.arange(S, dtype=np.float64) / max(SCALING, 1e-6)
    freqs = np.outer(t, inv_freq)                      # [S, HD/2]
    emb = np.concatenate((freqs, freqs), axis=-1)      # [S, HD]
    return np.cos(emb).astype(np.float32), np.sin(emb).astype(np.float32)


def _swz(w):
    kt = w.shape[0] // 128
    return np.ascontiguousarray(
        w.reshape(kt, 128, w.shape[1]).transpose(1, 0, 2).reshape(128, -1))


def _host_prep(x, Wq, Wk, Wv, Wo):
    cos, sin = _rope_tables()
    cosT2 = np.ascontiguousarray(np.tile(cos.T, (2, 1)))     # [128, S]
    sinT2 = np.ascontiguousarray(np.tile(sin.T, (2, 1)))
    sinT2[0::2, :] *= -1.0        # rotate_half sign folded into sin

    ii = np.arange(384)[None, :]
    jj = np.arange(128)[:, None]
    m = np.zeros((128, 384), dtype=np.float32)
    m[:, 0:128] += np.where(ii[:, 0:128] >= jj, 0.0, MASKVAL)
    m[:, 256:384] += np.where(ii[:, 256:384] - 256 < jj, 0.0, MASKVAL)
    m2 = np.ascontiguousarray(np.concatenate([m, m], axis=1))  # [128, 768]

    ones64 = np.ones((128, HPG * NKB), dtype=NPBF)

    in_maps = []
    for c in range(8):
        b, g = c // HG, c % HG
        gsl = slice(g * GD, (g + 1) * GD)
        in_maps.append({
            "xT": np.ascontiguousarray(x[b].T).astype(NPBF),
            "wq": _swz(Wq[gsl, :].T).astype(NPBF),
            "wk": _swz(Wk[gsl, :].T).astype(NPBF),
            "wv": _swz(Wv[gsl, :].T).astype(NPBF),
            "wo": _swz(Wo[:, gsl].T).astype(NPBF),
            "cos2": cosT2.astype(NPBF), "sin2": sinT2.astype(NPBF),
            "mask2": m2,
            "ones64": ones64,
        })
    return in_maps


def _run(inputs, trace=False, **kw):
    if "nc" not in _CACHE:
        _CACHE["nc"] = _build()
    in_maps = _host_prep(inputs["x"], inputs["Wq"], inputs["Wk"],
                         inputs["Wv"], inputs["Wo"])
    return run_bass_kernel_spmd(_CACHE["nc"], in_maps, list(range(8)),
                                trace=trace, **kw)


def kernel(x, Wq, Wk, Wv, Wo):
    res = _run({"x": x, "Wq": Wq, "Wk": Wk, "Wv": Wv, "Wo": Wo})
    out = np.zeros((B, S, D), dtype=np.float32)
    for c in range(8):
        out[c // HG] += np.asarray(res.results[c]["out"]).astype(np.float32)
    return out
